# revision 3
# baseline (speedup 1.0000x reference)
"""Causal single-head attention forward on 8 Trainium2 NeuronCores — v2.

Problem: x:(512,256,384) f32, Wq/Wk/Wv:(384,64) f32
  k = x@Wk; q = x@Wq; v = x@Wv
  wei = softmax(mask(q k^T / sqrt(384)))
  out = wei @ v                       -> (512, 256, 64) f32

v2 changes vs v1 (175us):
  - x loaded as plain f32 on the sync-engine HARDWARE DGE queue (v1 used a
    f32->bf16 cast DMA, which forces software descriptor generation on
    gpsimd: input path ran at ~180 GB/s and gpsimd was saturated).
  - PE transposes x straight from f32 (f32r stationary, bf16 identity
    stream) -> bf16 PSUM; no cast pass exists anywhere.
  - 4-stage software pipeline: at loop index k the engines work on
    transposes(k) | projections(k-1) | scores+exp+mask(k-2) | PV(k-3) |
    scale+store(k-4), so every cross-engine handoff has a full iteration
    of slack and PE never blocks on DVE/scalar copies.
  - merged instructions: one PSUM->SBUF copy for q,k,v ([128,1024]), one
    exp per pair, one reciprocal + one broadcast-multiply for the softmax
    normalization, two mask multiplies per pair.
  - projections write one PSUM tile: chain [Wq|Wk] -> rows 0:64 q^T
    (pre-scaled), rows 64:128 k^T at cols 0:512; chain Wv -> v^T at rows
    0:64 cols 512:1024.
"""

import os
from contextlib import ExitStack

import numpy as np

import concourse.bass as bass
import concourse.mybir as mybir
import concourse.tile as tile
from concourse import bacc
from concourse.masks import make_identity

B, T, C, H = 512, 256, 384, 64
N_CORES = 8
B_LOC = B // N_CORES          # 64 batches per core
MACRO = 2                     # batches per macro-iteration
N_MACRO = B_LOC // MACRO      # 32
NC_CHUNKS = C // 128          # 3
SCALE = float(C) ** -0.5

F32 = mybir.dt.float32
F32R = mybir.dt.float32r
BF16 = mybir.dt.bfloat16
EXP = mybir.ActivationFunctionType.Exp


def build_attention_kernel(ctx, tc, out_d, x_d, wq_d, wk_d, wv_d, reps=1,
                           variant="full"):
    nc = tc.nc
    pool = lambda *a, **k: ctx.enter_context(tc.tile_pool(*a, **k))

    # ---------------- constants / weights (once) ----------------
    cpool = pool(name="const", bufs=1)
    ident = cpool.tile([128, 128], BF16, tag="ident")
    make_identity(nc, ident[:, :])
    identf = cpool.tile([128, 128], F32, tag="identf")
    make_identity(nc, identf[:, :])

    # upper-triangular (incl diag) 0/1 mask in bf16: mask[j, i] = 1 if i >= j
    maskT = cpool.tile([128, 128], BF16, tag="maskT")
    nc.gpsimd.memset(maskT[:, :], 1.0)
    nc.gpsimd.affine_select(
        out=maskT[:, :],
        in_=maskT[:, :],
        compare_op=mybir.AluOpType.is_ge,
        fill=0.0,
        base=0,
        pattern=[[1, 128]],       # iota = -j + i  (channel j, free i)
        channel_multiplier=-1,
    )

    # weights: DRAM (384,64) -> staging f32 [128, 3*3, 64] -> packed bf16
    wst = cpool.tile([128, 3 * NC_CHUNKS, 64], F32, tag="wstage")
    for i, wd in enumerate((wq_d, wk_d, wv_d)):
        nc.sync.dma_start(
            wst[:, i * NC_CHUNKS:(i + 1) * NC_CHUNKS, :],
            wd.rearrange("(n p) h -> p n h", p=128),
        )
    # wkv packs [Wk | Wv] along the output dim; wq separate (pre-scaled).
    wkv = cpool.tile([128, NC_CHUNKS, 128], BF16, tag="wkv")
    nc.vector.tensor_copy(wkv[:, :, 0:64], wst[:, NC_CHUNKS:2 * NC_CHUNKS, :])
    nc.vector.tensor_copy(wkv[:, :, 64:128], wst[:, 2 * NC_CHUNKS:3 * NC_CHUNKS, :])
    wq = cpool.tile([128, NC_CHUNKS, 64], BF16, tag="wq")
    nc.vector.tensor_scalar_mul(wq[:, :, :], wst[:, 0:NC_CHUNKS, :], SCALE)

    # ---------------- pools ----------------
    xf_pool = pool(name="xf", bufs=4)       # f32 natural x [128, 2, 2, 384]
    xts_pool = pool(name="xts", bufs=6)     # bf16 xT chunks [128, 512]
    qkvs_pool = pool(name="qkvs", bufs=3)   # bf16 [qT;kT | vT] [128, 1024]
    vs_pool = pool(name="vs", bufs=3)       # bf16 v natural + ones col
    pt_pool = pool(name="pt", bufs=3)       # bf16 pT [128, 2, 384]
    rec_pool = pool(name="rec", bufs=2)     # f32 reciprocal [128, 2, 2]
    os_pool = pool(name="os", bufs=3)       # f32 out staging

    xtp_pool = pool(name="xtp", bufs=2, space="PSUM")    # xT psum bf16
    qkvp_pool = pool(name="qkvp", bufs=1, space="PSUM")  # [128,1024] f32
    vtp_pool = pool(name="vtp", bufs=1, space="PSUM")    # v natural bf16
    stp_pool = pool(name="stp", bufs=1, space="PSUM")    # sT [128, 2, 512] f32
    op_pool = pool(name="op", bufs=1, space="PSUM")      # out [128, 2, 2, 65]

    x_r = x_d.rearrange("(m b) (n p) c -> m p b n c", p=128, b=MACRO)
    out_rm = out_d.rearrange("(m b) (n p) h -> m p b n h", p=128, b=MACRO)

    if variant == "dma":
        # loads + stores only: measures the DMA floor of the v2 layout
        osb0 = os_pool.tile([128, MACRO, 2, 64], F32, tag="os")
        nc.vector.memset(osb0[:, :, :, :], 0.0)
        if reps > 1:
            rep_ctx = tc.For_i(0, reps, 1)
            rep_ctx.__enter__()
        for it in range(N_MACRO):
            xf = xf_pool.tile([128, MACRO, 2, C], F32, tag="xf")
            nc.sync.dma_start(xf[:, :, :, :], x_r[it])
            nc.sync.dma_start(out_rm[it], osb0[:, :, :, :])
        if reps > 1:
            rep_ctx.__exit__(None, None, None)
        return

    if reps > 1:
        rep_ctx = tc.For_i(0, reps, 1)
        rep_ctx.__enter__()

    # per-logical-index live tiles
    xf_t = {}
    xts_t = {}
    qkvs_t = {}
    vs_t = {}
    pt_t = {}
    stp_t = {}
    op_t = {}

    # prologue loads for xf(0), xf(1)
    for j in range(min(2, N_MACRO)):
        xf = xf_pool.tile([128, MACRO, 2, C], F32, tag="xf")
        nc.sync.dma_start(xf[:, :, :, :], x_r[j])
        xf_t[j] = xf

    for k in range(N_MACRO + 4):
        do_T = k < N_MACRO                    # transposes of pair k
        do_PJ = 0 <= k - 1 < N_MACRO          # projections of pair k-1
        do_AT = 0 <= k - 2 < N_MACRO          # scores/exp/mask/vT of k-2
        do_PV = 0 <= k - 3 < N_MACRO          # PV matmuls of k-3
        do_ST = 0 <= k - 4 < N_MACRO          # normalize + store of k-4

        # ---- sync engine: issue next load early ----
        if k + 2 < N_MACRO:
            xf = xf_pool.tile([128, MACRO, 2, C], F32, tag="xf")
            nc.sync.dma_start(xf[:, :, :, :], x_r[k + 2])
            xf_t[k + 2] = xf

        # ---- build the PE instruction list for this iteration ----
        # transposes (k): 12 tiles, chunk-major
        t_ops = []
        if do_T:
            xtps = []
            xf = xf_t[k]
            # bf16 truncation view: high 2 bytes of each little-endian f32
            xfb = xf[:, :, :, :].bitcast(BF16).rearrange(
                "p b t (c two) -> p b t c two", two=2)
            for c in range(NC_CHUNKS):
                xtp = xtp_pool.tile([128, 512], BF16, tag="xtp")
                xtps.append(xtp)
                for b in range(MACRO):
                    for t in range(2):
                        t_ops.append((
                            xtp[:, (b * 2 + t) * 128:(b * 2 + t) * 128 + 128],
                            xfb[:, b, t, c * 128:(c + 1) * 128, 1],
                        ))

        pj_ops = []
        if do_PJ:
            qkvp = qkvp_pool.tile([128, 1024], F32, tag="qkvp")
            xts = xts_t[k - 1]
            for c in range(NC_CHUNKS):
                pj_ops.append((qkvp[:, 0:512], wkv[:, c, :], xts[c][:, :],
                               c == 0, c == NC_CHUNKS - 1))
            for c in range(NC_CHUNKS):
                pj_ops.append((qkvp[0:64, 512:1024], wq[:, c, :], xts[c][:, :],
                               c == 0, c == NC_CHUNKS - 1))

        st_ops = []
        vt_ops = []
        if do_AT:
            qkvs = qkvs_t[k - 2]
            stp = stp_pool.tile([128, MACRO, 512], F32, tag="stp")
            stp_t[k - 2] = stp
            for b in range(MACRO):
                bc = b * 256
                st_ops.append((
                    stp[:, b, 0:256],
                    qkvs[0:64, bc:bc + 128],
                    qkvs[0:64, 512 + bc:512 + bc + 256],
                ))
                st_ops.append((
                    stp[:, b, 256:384],
                    qkvs[0:64, bc + 128:bc + 256],
                    qkvs[0:64, 512 + bc + 128:512 + bc + 256],
                ))
            vtp = vtp_pool.tile([128, 4, 64], BF16, tag="vtp")
            for q in range(4):
                vt_ops.append((
                    vtp[:, q, :],
                    qkvs[64:128, q * 128:(q + 1) * 128],
                    ident[64:128, 64:128],
                ))

        pv_ops = []
        if do_PV:
            pt = pt_t[k - 3]
            vsb = vs_t[k - 3]
            op = op_pool.tile([128, MACRO, 2, 65], F32, tag="op")
            op_t[k - 3] = op
            for b in range(MACRO):
                pv_ops.append((op[:, b, 0, :], pt[:, b, 0:128],
                               vsb[:, b * 2, :], True, True))
                pv_ops.append((op[:, b, 1, :], pt[:, b, 128:256],
                               vsb[:, b * 2, :], True, False))
                pv_ops.append((op[:, b, 1, :], pt[:, b, 256:384],
                               vsb[:, b * 2 + 1, :], False, True))

        # ---- emit PE stream: weave transposes between big matmuls ----
        # order: pj0..pj5 / sT / vT / PV woven with T0..T11
        big = ([("pj", o) for o in pj_ops]
               + [("st", o) for o in st_ops]
               + [("vt", o) for o in vt_ops]
               + [("pv", o) for o in pv_ops])
        tq = list(t_ops)
        emitted_t = 0
        # emit one transpose after each big op; extras at the end
        for kind, o in big:
            if kind == "pj":
                nc.tensor.matmul(o[0], o[1], o[2], start=o[3], stop=o[4])
            elif kind == "st":
                nc.tensor.matmul(o[0], o[1], o[2], start=True, stop=True)
            elif kind == "vt":
                nc.tensor.transpose(o[0], o[1], o[2])
            else:
                nc.tensor.matmul(o[0], o[1], o[2], start=o[3], stop=o[4])
            if tq:
                o2 = tq.pop(0)
                nc.tensor.transpose(o2[0], o2[1], ident[:, :])
                emitted_t += 1
        while tq:
            o2 = tq.pop(0)
            nc.tensor.transpose(o2[0], o2[1], ident[:, :])

        # ---- scalar engine: xts chunk 0/1 copies, exp ----
        if do_T:
            xts_t[k] = []
            for c in range(NC_CHUNKS):
                xt = xts_pool.tile([128, 512], BF16, tag="xts")
                xts_t[k].append(xt)
            nc.scalar.copy(xts_t[k][0][:, :], xtps[0][:, :])
            nc.scalar.copy(xts_t[k][1][:, :], xtps[1][:, :])
        if do_AT:
            pt = pt_pool.tile([128, MACRO, 384], BF16, tag="pt")
            pt_t[k - 2] = pt
            nc.scalar.activation(pt[:, :, :], stp_t[k - 2][:, :, 0:384], EXP)

        # ---- gpsimd: causal masks on the two diagonal blocks ----
        if do_AT:
            pt = pt_t[k - 2]
            mb = maskT[:, :].unsqueeze(1).broadcast_to((128, MACRO, 128))
            nc.gpsimd.tensor_mul(pt[:, :, 0:128], pt[:, :, 0:128], mb)
            nc.gpsimd.tensor_mul(pt[:, :, 256:384], pt[:, :, 256:384], mb)

        # ---- vector engine ----
        if do_ST:
            op = op_t[k - 4]
            rec = rec_pool.tile([128, MACRO, 2], F32, tag="rec")
            nc.vector.reciprocal(rec[:, :, :], op[:, :, :, 64])
            osb = os_pool.tile([128, MACRO, 2, 64], F32, tag="os")
            nc.vector.tensor_mul(
                osb[:, :, :, :],
                op[:, :, :, 0:64],
                rec[:, :, :].unsqueeze(3).broadcast_to((128, MACRO, 2, 64)),
            )
            nc.sync.dma_start(out_rm[k - 4], osb[:, :, :, :])
        if do_PJ:
            qkvs = qkvs_pool.tile([128, 1024], BF16, tag="qkvs")
            qkvs_t[k - 1] = qkvs
            nc.vector.tensor_copy(qkvs[:, 0:512], qkvp[:, 0:512])
            nc.vector.tensor_copy(qkvs[0:64, 512:1024], qkvp[0:64, 512:1024])
        if do_T:
            nc.vector.tensor_copy(xts_t[k][2][:, :], xtps[2][:, :])
        if do_AT:
            vsb = vs_pool.tile([128, 4, 65], BF16, tag="vs")
            vs_t[k - 2] = vsb
            nc.vector.memset(vsb[:, :, 64], 1.0)
            nc.vector.tensor_copy(vsb[:, :, 0:64], vtp[:, :, :])

        # drop dead references
        for d, off in ((xf_t, 0), (xts_t, -1), (qkvs_t, -2), (stp_t, -2),
                       (vs_t, -3), (pt_t, -3), (op_t, -4)):
            d.pop(k + off - 1, None)

    if reps > 1:
        rep_ctx.__exit__(None, None, None)


_CACHED = {}


def _build(reps=1, variant="full"):
    key = (reps, variant)
    if key in _CACHED:
        return _CACHED[key]
    nc = bacc.Bacc(
        "TRN2",
        target_bir_lowering=False,
        debug=False,
        num_devices=N_CORES,
    )
    x_d = nc.dram_tensor("x", [B_LOC, T, C], F32, kind="ExternalInput").ap()
    wq_d = nc.dram_tensor("Wq", [C, H], F32, kind="ExternalInput").ap()
    wk_d = nc.dram_tensor("Wk", [C, H], F32, kind="ExternalInput").ap()
    wv_d = nc.dram_tensor("Wv", [C, H], F32, kind="ExternalInput").ap()
    out_d = nc.dram_tensor("out", [B_LOC, T, H], F32, kind="ExternalOutput").ap()
    with tile.TileContext(nc) as tc, ExitStack() as ctx:
        build_attention_kernel(
            ctx, tc, out_d, x_d, wq_d, wk_d, wv_d, reps=reps, variant=variant
        )
    nc.compile()
    _CACHED[key] = nc
    return nc


_RUNNER = {}


def _get_runner(reps=1, variant="full"):
    """Persistent jitted SPMD executor (compiles/loads the NEFF once)."""
    key = (reps, variant)
    if key in _RUNNER:
        return _RUNNER[key]

    import jax
    from jax.sharding import Mesh, PartitionSpec
    from jax.experimental.shard_map import shard_map
    from concourse import bass2jax

    nc = _build(reps, variant)
    bass2jax.install_neuronx_cc_hook()

    partition_name = (
        nc.partition_id_tensor.name if nc.partition_id_tensor else None
    )
    in_names, out_names, out_avals = [], [], []
    for alloc in nc.m.functions[0].allocations:
        if not isinstance(alloc, mybir.MemoryLocationSet):
            continue
        name = alloc.memorylocations[0].name
        if alloc.kind == "ExternalInput":
            if name != partition_name:
                in_names.append(name)
        elif alloc.kind == "ExternalOutput":
            out_names.append(name)
            out_avals.append(
                jax.core.ShapedArray(
                    tuple(alloc.tensor_shape), mybir.dt.np(alloc.dtype)
                )
            )
    n_params = len(in_names)
    all_in_names = in_names + out_names
    if partition_name is not None:
        all_in_names = all_in_names + [partition_name]

    def _body(*args):
        operands = list(args)
        if partition_name is not None:
            operands.append(bass2jax.partition_id_tensor())
        outs = bass2jax._bass_exec_p.bind(
            *operands,
            out_avals=tuple(out_avals),
            in_names=tuple(all_in_names),
            out_names=tuple(out_names),
            lowering_input_output_aliases=(),
            sim_require_finite=True,
            sim_require_nnan=True,
            nc=nc,
        )
        return tuple(outs)

    devices = jax.devices()[:N_CORES]
    mesh = Mesh(np.asarray(devices), ("core",))
    fn = jax.jit(
        shard_map(
            _body,
            mesh=mesh,
            in_specs=(PartitionSpec("core"),) * (n_params + len(out_names)),
            out_specs=(PartitionSpec("core"),) * len(out_names),
            check_rep=False,
        ),
        keep_unused=True,
    )
    zero_outs = [
        np.zeros((N_CORES * a.shape[0], *a.shape[1:]), a.dtype) for a in out_avals
    ]
    _RUNNER[key] = (fn, in_names, out_names, out_avals, zero_outs)
    return _RUNNER[key]


def _global_inputs(x, Wk, Wq, Wv):
    """Concatenated per-core inputs keyed by BIR input name."""
    reps = {
        "x": np.ascontiguousarray(x, dtype=np.float32),
        "Wq": np.tile(np.asarray(Wq, np.float32), (N_CORES, 1)),
        "Wk": np.tile(np.asarray(Wk, np.float32), (N_CORES, 1)),
        "Wv": np.tile(np.asarray(Wv, np.float32), (N_CORES, 1)),
    }
    return reps


def kernel(x, Wk, Wq, Wv):
    x = np.asarray(x, dtype=np.float32)
    fn, in_names, out_names, out_avals, zero_outs = _get_runner()
    gi = _global_inputs(x, Wk, Wq, Wv)
    args = [gi[n] for n in in_names] + zero_outs
    outs = fn(*args)
    out = np.asarray(outs[out_names.index("out")])
    return out.astype(np.float32)
